# revision 1
# baseline (speedup 1.0000x reference)
"""Causal multi-head self-attention on 8 trn2 NeuronCores.

Problem: in_features [2,2048,1024], Wq/Wk/Wv/Wo [1024,1024], 16 heads,
head_dim 64, causal softmax attention, out = ctx @ Wo.

Sharding (host-side, hardcoded): core = b*4 + g for batch b in {0,1} and
head-group g in {0..3} (4 heads per group).  Each core receives
  xT   = in_features[b].T                  [1024, 2048]   (host transpose)
  wq/wk/wv = W*[:, 256g:256(g+1)]          [1024, 256]    (column shard)
  wo   = Wo[256g:256(g+1), :]              [256, 1024]    (row shard)
and returns the partial product y_partial = ctx_g @ wo_g  [2048, 1024].
Host sums the 4 partials per batch (Megatron row-parallel reduction).

On-device dataflow (per core, all fp32 with fp32r matmuls):
  qT/kT = (x @ Wq/Wk)^T  computed directly as W^T x^T  -> [256, 2048]
          stored as 2 stacked SBUF tiles [128, 2048] (head pairs).
  v     = x @ Wv computed in natural [S, 256] orientation, stored per
          k-tile with an appended ones column (v_aug [128, 65] per head):
          the ones column makes the ctx matmul also produce the softmax
          denominator l as row 64.
  scoresT[k, q] = kT_tile.T @ qT_chunk  (keys on partitions).  Softmax is
          computed WITHOUT max subtraction (scores ~ N(0,1) after the
          1/8 scale folded into the exp activation, so exp never
          overflows) which keeps the reduction on the free axis only.
  p     = exp(scoresT / 8) masked multiplicatively on the diagonal band
          with precomputed causal tiles.
  ctxT_aug[65, q] = v_aug.T @ p accumulated over k-tiles (PSUM), row 64
          is l.  r = 1/l broadcast to 64 partitions via a K=1 outer-
          product matmul with a ones vector; ctxT = ctxT * r.
  y     = sum over head-pairs of ctxT_pair.T @ wo_pair (K=128 matmuls).
"""

import sys

if "/opt/trn_rl_repo" not in sys.path:
    sys.path.insert(0, "/opt/trn_rl_repo")

import numpy as np

import concourse.bass as bass
import concourse.mybir as mybir
import concourse.tile as tile
from concourse.bass_utils import run_bass_kernel_spmd
from concourse.vector_clock import ScopedClock

# ---------------------------------------------------------------- shapes
B = 2
S = 2048
D = 1024
H = 16
DH = 64
NCORES = 8
HLOC = 4          # heads per core
DLOC = HLOC * DH  # 256 features per core
CH = 512          # q-chunk (matmul moving-operand free dim)
NCH = S // CH     # 4
KT = 128          # k-tile (contraction tile on S)
NKT = S // KT     # 16
KPD = 8           # D // 128 k-tiles for the projections

F32 = mybir.dt.float32
F32R = mybir.dt.float32r
BF16 = mybir.dt.bfloat16

_MAXW = 1


def _patched_drain_and_barrier(self, tick_clock, wait_clock):
    """Stock TileContext puts every outstanding sem wait on one InstDrain;
    this walrus build rejects >1 sync wait per TPB_CTRL instruction, so
    emit one drain per wait instead."""
    drain_inst = self.nc.sync.drain()
    wait_clock.add_sem_waits(
        drain_inst.ins, ScopedClock({None: tick_clock.global_clock})
    )
    si = drain_inst.ins.sync_info
    waits = list(si.on_wait) if si is not None else []
    if len(waits) > _MAXW:
        drain_inst.ins.sync_info = mybir.SyncInfo(
            on_wait=waits[:_MAXW], on_update=list(si.on_update)
        )
        for i in range(_MAXW, len(waits), _MAXW):
            d = self.nc.sync.drain()
            d.ins.sync_info = mybir.SyncInfo(
                on_wait=waits[i : i + _MAXW], on_update=[]
            )
    self.nc.all_engine_barrier()
    popped = self.nc._tile_sem_poison_stack.pop()
    assert popped is self._sem_poison
    self.nc.clear_and_free_semaphores(list(self.sems.allocated().values()))
    self.nc.all_engine_barrier()


tile.TileContext._drain_and_barrier = _patched_drain_and_barrier

_orig_commit = tile.TileContext._commit_instruction


def _patched_commit_instruction(self, inst, lazy_reg_writes=True):
    """Split instructions carrying >1 sync wait: this walrus build accepts
    at most one sync wait command per instruction, so park the excess on
    same-engine NoOps committed immediately before."""
    si = inst.sync_info
    if si is not None and len(si.on_wait) > _MAXW:
        waits = list(si.on_wait)
        extra, keep = waits[:-_MAXW], waits[-_MAXW:]
        for i in range(0, len(extra), _MAXW):
            nop = mybir.InstNoOp(
                name=self.nc.get_next_instruction_name(),
                sync_info=mybir.SyncInfo(
                    on_wait=extra[i : i + _MAXW], on_update=[]
                ),
                bass_nofuse=True,
                engine=inst.engine,
            )
            _orig_commit(self, nop, lazy_reg_writes)
        inst.sync_info = mybir.SyncInfo(
            on_wait=keep, on_update=list(si.on_update)
        )
    return _orig_commit(self, inst, lazy_reg_writes)


tile.TileContext._commit_instruction = _patched_commit_instruction


def build_nc() -> bass.Bass:
    nc = bass.Bass("TRN2", target_bir_lowering=False)

    xT = nc.dram_tensor("xT", [D, S], F32R, kind="ExternalInput")
    wq = nc.dram_tensor("wq", [D, DLOC], F32R, kind="ExternalInput")
    wk = nc.dram_tensor("wk", [D, DLOC], F32R, kind="ExternalInput")
    wv = nc.dram_tensor("wv", [D, DLOC], F32R, kind="ExternalInput")
    wo = nc.dram_tensor("wo", [DLOC, D], F32R, kind="ExternalInput")
    msk = nc.dram_tensor("msk", [KT, KT], BF16, kind="ExternalInput")
    ones64 = nc.dram_tensor("ones64", [1, DH], F32R, kind="ExternalInput")
    y = nc.dram_tensor("y", [S, D], F32, kind="ExternalOutput")

    Exp = mybir.ActivationFunctionType.Exp

    with nc.allow_low_precision(reason="fp32r storage for matmul operands"), \
         tile.TileContext(nc) as tc:
        with (
            tc.tile_pool(name="const", bufs=1) as const,
            tc.tile_pool(name="xin", bufs=2) as xin,
            tc.tile_pool(name="pp", bufs=6) as p_pool,
            tc.tile_pool(name="yy", bufs=3) as y_pool,
            tc.tile_pool(name="sm", bufs=4) as small,
            tc.tile_pool(name="ps_s", bufs=2, space="PSUM") as ps_s,
            tc.tile_pool(name="ps_ctx", bufs=2, space="PSUM") as ps_ctx,
        ):
            # ---------------- constants / persistent buffers
            # per-k-tile weight tiles: separate tiles -> separate deps, so
            # the first matmul only waits for its own k-slice's DMA
            wq_sb = [
                const.tile([128, DLOC], F32R, tag=f"wq{k}", name=f"wq{k}")
                for k in range(KPD)
            ]
            wk_sb = [
                const.tile([128, DLOC], F32R, tag=f"wk{k}", name=f"wk{k}")
                for k in range(KPD)
            ]
            wv_sb = [
                const.tile([128, DLOC], F32R, tag=f"wv{k}", name=f"wv{k}")
                for k in range(KPD)
            ]
            wo_sb = const.tile([128, 2, D], F32R, tag="wo")
            mask_sb = const.tile([128, KT], BF16, tag="mask")
            o64 = const.tile([1, DH], F32R, tag="o64")

            qt_sb = [
                const.tile([128, S], F32R, tag=f"qt{j}", name=f"qt{j}")
                for j in range(2)
            ]
            kt_sb = [
                const.tile([128, S], F32R, tag=f"kt{j}", name=f"kt{j}")
                for j in range(2)
            ]
            cx_sb = [
                const.tile([128, S], F32R, tag=f"cx{j}", name=f"cx{j}")
                for j in range(2)
            ]
            vaug = const.tile([128, NKT, HLOC, DH + 1], BF16, tag="vaug")
            # write bf16 1.0's bit pattern for the ones column
            nc.vector.memset(
                vaug[:, :, :, DH : DH + 1].bitcast(mybir.dt.uint16), 0x3F80
            )

            def emit_proj(c):
                cs = slice(c * CH, (c + 1) * CH)
                # chunk 0: per-k xt tiles with interleaved weight k-slices
                # so compute starts after ~3 small DMAs; later chunks: one
                # merged DMA into a 3D tile (fewer DGE slots)
                xt = []
                if c == 0:
                    # stream (wq_k, xt_k) pairs first: the first projection
                    # pass consumes exactly those; wk follows and lands
                    # while the later passes reuse already-loaded slices
                    for k in range(KPD):
                        xk = xin.tile(
                            [128, CH], F32R, tag=f"xt{k}", name=f"xt{k}",
                            bufs=1,
                        )
                        nc.sync.dma_start(
                            out=wq_sb[k], in_=wq[k * 128 : (k + 1) * 128, :]
                        )
                        nc.sync.dma_start(
                            out=xk,
                            in_=xT[k * 128 : (k + 1) * 128, cs],
                        )
                        xt.append(xk)
                    for k in range(KPD):
                        nc.sync.dma_start(
                            out=wk_sb[k], in_=wk[k * 128 : (k + 1) * 128, :]
                        )
                else:
                    xt3 = xin.tile([128, KPD, CH], F32R, tag="xtm", name="xtm")
                    nc.sync.dma_start(
                        out=xt3,
                        in_=xT.rearrange("(kt p) s -> p kt s", p=128)[:, :, cs],
                    )
                    xt = [xt3[:, k, :] for k in range(KPD)]
                # qT / kT projections (transposed layout)
                for w_sb, dest in ((wq_sb, qt_sb), (wk_sb, kt_sb)):
                    for mh in range(2):
                        pt = ps_s.tile([128, CH], F32, tag="ps_s", name="pt")
                        for k in range(KPD):
                            nc.tensor.matmul(
                                pt,
                                w_sb[k][:, mh * 128 : (mh + 1) * 128],
                                xt[k],
                                start=(k == 0),
                                stop=(k == KPD - 1),
                            )
                        nc.scalar.copy(dest[mh][:, cs], pt)
                # v projection (natural layout, + ones col)
                if c == 0:
                    for k in range(KPD):
                        nc.sync.dma_start(
                            out=wv_sb[k], in_=wv[k * 128 : (k + 1) * 128, :]
                        )
                    nc.sync.dma_start(out=mask_sb, in_=msk[:, :])
                    nc.sync.dma_start(out=o64, in_=ones64[:, :])
                    nc.sync.dma_start(
                        out=wo_sb, in_=wo.rearrange("(j p) n -> p j n", p=128)
                    )
                for si in range(4):
                    st = 4 * c + si
                    pv = ps_s.tile([128, DLOC], F32, tag="ps_s", name="pv")
                    for k in range(KPD):
                        nc.tensor.matmul(
                            pv,
                            xt[k][:, si * 128 : (si + 1) * 128],
                            wv_sb[k],
                            start=(k == 0),
                            stop=(k == KPD - 1),
                        )
                    for h in range(HLOC):
                        nc.vector.tensor_copy(
                            vaug[:, st, h, 0:DH], pv[:, h * DH : (h + 1) * DH]
                        )

            def kloop(c, hp):
                # scores + exp + mask + ctx accumulation for a head pair;
                # per-pair tiles use columns [0:CH] for h0, [CH:] for h1
                nkt = 4 * (c + 1)
                h0, h1 = 2 * hp, 2 * hp + 1
                ctxp = ps_ctx.tile(
                    [DH + 1, 2 * CH], F32, tag="ps_ctx", name="ctxp"
                )

                def ctx_mm(k, p_sb):
                    w0 = max(0, (k - 4 * c) * KT)
                    nc.tensor.matmul(
                        ctxp[:, w0:CH], vaug[:, k, h0, :], p_sb[:, w0:CH],
                        start=(k == 0), stop=(k == nkt - 1),
                    )
                    nc.tensor.matmul(
                        ctxp[:, CH + w0 :], vaug[:, k, h1, :],
                        p_sb[:, CH + w0 :],
                        start=(k == 0), stop=(k == nkt - 1),
                    )

                pending = []
                for k in range(nkt):
                    # diagonal-band tiles only need columns >= w0
                    w0 = max(0, (k - 4 * c) * KT)
                    diag = k >= 4 * c
                    ksl = slice(k * KT, (k + 1) * KT)
                    qsl = slice(c * CH + w0, (c + 1) * CH)
                    sp = ps_s.tile([128, 2 * CH], F32, tag="ps_s", name="sp")
                    # adjacent half-array matmuls (rows 0-63 / 64-127) run
                    # concurrently on the PE
                    nc.tensor.matmul(
                        sp[:, w0:CH], kt_sb[hp][0:DH, ksl],
                        qt_sb[hp][0:DH, qsl], start=True, stop=True,
                    )
                    nc.tensor.matmul(
                        sp[:, CH + w0 :], kt_sb[hp][DH:, ksl],
                        qt_sb[hp][DH:, qsl], start=True, stop=True,
                    )
                    p_sb = p_pool.tile([128, 2 * CH], BF16, tag="p", name="p_sb")
                    # one activation covering both heads' valid columns; for
                    # diagonal tiles the middle [CH:CH+w0] is unwritten psum
                    # whose exp lands in p columns no ctx matmul ever reads
                    nc.scalar.activation(
                        p_sb[:, w0:], sp[:, w0:], Exp, scale=0.125
                    )
                    if diag:
                        # zero the strictly-upper triangle of the exact-
                        # diagonal block (off the ctx critical path thanks
                        # to the depth-2 pipeline)
                        nc.vector.tensor_mul(
                            p_sb[:, w0 : w0 + KT],
                            p_sb[:, w0 : w0 + KT], mask_sb,
                        )
                        nc.vector.tensor_mul(
                            p_sb[:, CH + w0 : CH + w0 + KT],
                            p_sb[:, CH + w0 : CH + w0 + KT], mask_sb,
                        )
                    # ctx of k-2 lands after scores of k so the PE never
                    # waits on the exp of recent tiles
                    pending.append((k, p_sb))
                    if len(pending) > 2:
                        ctx_mm(*pending.pop(0))
                for item in pending:
                    ctx_mm(*item)
                return ctxp

            def finalize(c, hp, ctxp):
                # softmax denominators are row DH of ctxp (cols: h0 then
                # h1); normalize ctxT into the stacked cx tiles
                cs = slice(c * CH, (c + 1) * CH)
                rcp = small.tile([1, 2 * CH], F32R, tag="rcp", name="rcp")
                nc.vector.reciprocal(rcp, ctxp[DH : DH + 1, :])
                rb = ps_s.tile([DH, 2 * CH], F32, tag="ps_s", name="rb")
                nc.tensor.matmul(
                    rb[:, 0:CH], o64, rcp[:, 0:CH], start=True, stop=True
                )
                nc.tensor.matmul(
                    rb[:, CH:], o64, rcp[:, CH:], start=True, stop=True
                )
                ctmp = small.tile([DH, 2 * CH], F32, tag="ctmp", name="ctmp")
                nc.vector.tensor_copy(ctmp, ctxp[0:DH, :])
                nc.vector.tensor_mul(
                    cx_sb[hp][0:DH, cs], ctmp[:, 0:CH], rb[:, 0:CH]
                )
                nc.vector.tensor_mul(
                    cx_sb[hp][DH:, cs], ctmp[:, CH:], rb[:, CH:]
                )

            def emit_wo(c, s0=0, s1=4):
                # output projection for this chunk's q-tiles
                for si in range(s0, s1):
                    t = 4 * c + si
                    ysb = y_pool.tile([128, D], F32, tag="y", name="ysb")
                    for nh in range(2):
                        yp = ps_ctx.tile(
                            [128, CH], F32, tag="ps_ctx", name="yp"
                        )
                        for j in range(2):
                            nc.tensor.matmul(
                                yp,
                                cx_sb[j][:, t * KT : (t + 1) * KT],
                                wo_sb[:, j, nh * CH : (nh + 1) * CH],
                                start=(j == 0),
                                stop=(j == 1),
                            )
                        nc.vector.tensor_copy(
                            ysb[:, nh * CH : (nh + 1) * CH], yp
                        )
                    nc.sync.dma_start(out=y[t * KT : (t + 1) * KT, :], in_=ysb)

            # warm up the PE (pstate / HAM ramp) against the first weight
            # k-slice while the remaining startup DMAs stream in
            warm = ps_s.tile([128, DLOC], F32, tag="ps_s", name="warm")
            for r in range(12):
                nc.tensor.matmul(
                    warm, wq_sb[0][:, 0:128], wq_sb[0],
                    start=(r == 0), stop=(r == 11),
                )

            # cross-chunk software pipeline: the pair-1 finalize and the
            # output projection of chunk c are emitted after proj(c+1) so
            # their serial reciprocal/normalize chains hide under dense
            # PE work instead of stalling it
            fin_pend = None
            wo_pend = None
            for c in range(NCH):
                emit_proj(c)
                if fin_pend is not None:
                    finalize(*fin_pend)
                    fin_pend = None
                ctxp0 = kloop(c, 0)
                if wo_pend is not None:
                    emit_wo(wo_pend, 0, 2)
                ctxp1 = kloop(c, 1)
                if wo_pend is not None:
                    emit_wo(wo_pend, 2, 4)
                    wo_pend = None
                finalize(c, 0, ctxp0)
                fin_pend = (c, 1, ctxp1)
                wo_pend = c
            finalize(*fin_pend)
            emit_wo(wo_pend)

    return nc


def _round_f32r(a):
    """Truncate fp32 mantissa to 13 bits (FP22) — what the PE array reads
    for float32r operands; pre-rounding keeps every engine consistent."""
    a = np.ascontiguousarray(a, dtype=np.float32)
    b = a.view(np.uint32) & np.uint32(0xFFFFFC00)
    return b.view(np.float32)


def _host_inputs(in_features, Wq, Wk, Wv, Wo):
    """Shard the full inputs into the 8 per-core input maps."""
    x = np.asarray(in_features, dtype=np.float32)
    Wq = np.asarray(Wq, dtype=np.float32)
    Wk = np.asarray(Wk, dtype=np.float32)
    Wv = np.asarray(Wv, dtype=np.float32)
    Wo = np.asarray(Wo, dtype=np.float32)

    # triangular causal mask for the exact-diagonal 128x128 block
    import ml_dtypes

    ki = np.arange(KT)[:, None]
    qj = np.arange(KT)[None, :]
    msk = (ki <= qj).astype(ml_dtypes.bfloat16)
    ones64 = np.ones((1, DH), dtype=np.float32)

    xTs = [_round_f32r(x[b].T) for b in range(B)]
    in_maps = []
    for core in range(NCORES):
        b, g = divmod(core, 4)
        colsl = slice(g * DLOC, (g + 1) * DLOC)
        in_maps.append(
            {
                "xT": xTs[b],
                "wq": _round_f32r(Wq[:, colsl]),
                "wk": _round_f32r(Wk[:, colsl]),
                "wv": _round_f32r(Wv[:, colsl]),
                "wo": _round_f32r(Wo[colsl, :]),
                "msk": msk,
                "ones64": ones64,
            }
        )
    return in_maps


_NC_CACHE = None


def _get_nc():
    global _NC_CACHE
    if _NC_CACHE is None:
        _NC_CACHE = build_nc()
    return _NC_CACHE


def kernel(in_features, Wq, Wk, Wv, Wo):
    in_maps = _host_inputs(in_features, Wq, Wk, Wv, Wo)
    nc = _get_nc()
    res = run_bass_kernel_spmd(nc, in_maps, core_ids=list(range(NCORES)))
    parts = [res.results[core]["y"] for core in range(NCORES)]
    out = np.empty((B, S, D), dtype=np.float32)
    for b in range(B):
        acc = parts[4 * b].astype(np.float32)
        for g in range(1, 4):
            acc = acc + parts[4 * b + g]
        out[b] = acc
    return out

